# revision 11
# baseline (speedup 1.0000x reference)
"""Trainium2 Bass kernel for nn_BoundMemUpdate (spiking membrane update).

Computes, for x:[T,B,D], W:[D,D], b:[D]:
    mm[t] = x[t] @ W.T + b
    m[t] = mm[t] + m[t-1] * (1 - s[t-1]) * 0.5
    s[t] = (m[t] >= 1.0)
Returns (m, s), each [T, B, D] float32.

Sharding: output-dim (D_out) sharded 8 ways across cores (512 each);
x replicated, W/b sharded by rows. The recurrence is per-neuron
elementwise, so no cross-core communication is needed.

Matmul: single-term fp16. x and W are rounded to fp16; the PE forms
exact fp16 products with fp32 accumulation, so the only error is the
input rounding (~2^-11 relative per operand). On the fixed seed-0
problem instance this measures 6.5e-3 concatenated relative error
(506 spike flips of 8.4M) against the fp32 reference -- 3x inside
the 2e-2 gate. W is the PE-stationary operand and x the moving one,
giving output tiles [o_part, (t,b)] and 512 matmuls of 518 cycles
per core (~111 us of PE streaming at 2.4 GHz).

Schedule: 4 passes over t-pairs, 4 o-block PSUM chains per pass with
K=4096 contraction, double-generation PSUM banks (8 total) so pass
boundaries never wait on drains. The vector engine fuses the drain
with the temporal recurrence; the bias enters through the scalar
engine's per-partition bias port (d-state starts at b and is
re-biased every step). x is stored pass-interleaved in DRAM
([pass][128][kt][t][b]) so every DMA is contiguous on both sides;
pass 0 streams k-outer so the PE consumes (W, x) chunks as they
land, and a short warmup matmul chain ramps the PE clock during the
cold 8 MB load. Outputs go out as m:fp16 / s:fp8 in [t, o, b]
layout; the host widens and transposes during the final gather.
"""
import os
import numpy as np

import concourse.bass as bass
import concourse.mybir as mybir
from concourse import bacc
from concourse.tile import TileContext
from concourse.bass_utils import run_bass_kernel_spmd

T, B, D = 8, 256, 4096
N_CORES = 8
O_SHARD = D // N_CORES   # 512
KT = D // 128            # 32 fp16 k-tiles
OB = O_SHARD // 128      # 4 output blocks
NPASS = T // 2           # 4 t-pair passes
ALPHA = 0.5
M_TH = 1.0
WARM_MM = int(os.environ.get("BMU_WARM", "32"))
PSB = int(os.environ.get("BMU_PSB", "2"))       # psum pool bufs
XCH = int(os.environ.get("BMU_XCH", "8"))       # x DMA chunks per pass
OFFLOAD = os.environ.get("BMU_OFFLOAD", "1") == "1"  # s8->gpsimd, m16->scalar

F16 = mybir.dt.float16
F8 = mybir.dt.float8e4
F32 = mybir.dt.float32
NP_F16 = np.float16

_cache = {}


def _build_kernel(reps: int = 1):
    nc = bacc.Bacc("TRN2", target_bir_lowering=False, debug=False,
                   num_devices=N_CORES)

    NF16 = KT * 2 * B        # per-pass x free size (16384)

    wh16_d = nc.dram_tensor("wh16", [128, KT * O_SHARD], F16,
                            kind="ExternalInput").ap()
    xh16_d = nc.dram_tensor("xh16", [NPASS, 128, NF16], F16,
                            kind="ExternalInput").ap()
    bcol_d = nc.dram_tensor("bcol", [128, OB], F32,
                            kind="ExternalInput").ap()
    m_d = nc.dram_tensor("m_out", [T, O_SHARD, B], F16,
                         kind="ExternalOutput").ap()
    s_d = nc.dram_tensor("s_out", [T, O_SHARD, B], F8,
                         kind="ExternalOutput").ap()

    with TileContext(nc) as tc:
        with tc.tile_pool(name="wpool", bufs=1) as wpool, \
             tc.tile_pool(name="xhpool", bufs=2) as xhpool, \
             tc.tile_pool(name="cpool", bufs=1) as cpool, \
             tc.tile_pool(name="mpool", bufs=4) as mpool, \
             tc.tile_pool(name="opool", bufs=4) as opool, \
             tc.tile_pool(name="spool", bufs=4) as spool, \
             tc.tile_pool(name="upool", bufs=3) as upool, \
             tc.tile_pool(name="psum", bufs=PSB, space="PSUM") as psum_pool:

            whs = wpool.tile([128, KT * O_SHARD], F16, name="whs")
            bcol_t = cpool.tile([128, OB], F32)
            d_t = [cpool.tile([128, B], F32, name=f"d{ob}") for ob in range(OB)]
            warm_t = cpool.tile([128, 2 * B], F16, name="warm")
            dump_t = cpool.tile([128, 16], F32, name="dump")

            xsz = NF16 // XCH
            WCH = 8
            wsz = KT * O_SHARD // WCH

            # pass-0 load, interleaved in PE consumption order (k-outer
            # pass 0 walks kt 0..31 across o-blocks, so pair (wh16, xh16)
            # chunks by kt range).
            xh0 = xhpool.tile([128, NF16], F16, tag="xh")
            nc.sync.dma_start(out=bcol_t, in_=bcol_d)
            for c in range(XCH):
                wsl = slice(c * wsz, (c + 1) * wsz)
                nc.sync.dma_start(out=whs[:, wsl], in_=wh16_d[:, wsl])
                xsl = slice(c * xsz, (c + 1) * xsz)
                nc.sync.dma_start(out=xh0[:, xsl], in_=xh16_d[0][:, xsl])

            wh_k = whs.rearrange("p (kt o) -> p kt o", kt=KT)

            def warmup():
                if WARM_MM <= 0:
                    return
                nc.vector.memset(warm_t, 0.0)
                wp = psum_pool.tile([128, 2 * B], F32, tag="hi3",
                                    name="warmpsum")
                for i in range(WARM_MM):
                    nc.tensor.matmul(wp, warm_t[:, :128], warm_t,
                                     start=(i == 0), stop=(i == WARM_MM - 1))
                nc.vector.tensor_copy(out=dump_t, in_=wp[:, :16])

            def body(first=None, warm=False):
                for ob in range(OB):
                    nc.vector.memset(d_t[ob], 0.0)
                    nc.scalar.add(d_t[ob], d_t[ob], bcol_t[:, ob:ob + 1])
                if warm:
                    warmup()
                for p in range(NPASS):
                    if p == 0 and first is not None:
                        xh = first
                    else:
                        xh = xhpool.tile([128, NF16], F16, tag="xh")
                        for c in range(XCH):
                            xsl = slice(c * xsz, (c + 1) * xsz)
                            nc.sync.dma_start(out=xh[:, xsl],
                                              in_=xh16_d[p][:, xsl])

                    xh_kv = xh.rearrange("p (kt n) -> p kt n", kt=KT)

                    hi_t = [psum_pool.tile([128, 2 * B], F32, tag=f"hi{ob}",
                                           name=f"hi{p}_{ob}")
                            for ob in range(OB)]

                    def mm_hi(ob, kt):
                        osl = slice(ob * 128, (ob + 1) * 128)
                        nc.tensor.matmul(hi_t[ob], wh_k[:, kt, osl],
                                         xh_kv[:, kt, :],
                                         start=(kt == 0), stop=(kt == KT - 1))

                    def drain(ob):
                        osl = slice(ob * 128, (ob + 1) * 128)
                        for ti in range(2):
                            t = 2 * p + ti
                            bsl = slice(ti * B, (ti + 1) * B)
                            m_sb = mpool.tile([128, B], F32, tag="m")
                            nc.vector.tensor_add(out=m_sb,
                                                 in0=hi_t[ob][:, bsl],
                                                 in1=d_t[ob])
                            s_sb = spool.tile([128, B], F8, tag="s")
                            m16_sb = opool.tile([128, B], F16, tag="m16")
                            if OFFLOAD:
                                nc.gpsimd.tensor_scalar(
                                    out=s_sb, in0=m_sb, scalar1=M_TH,
                                    scalar2=None, op0=mybir.AluOpType.is_ge)
                                nc.scalar.copy(m16_sb, m_sb)
                            else:
                                nc.vector.tensor_scalar(
                                    out=s_sb, in0=m_sb, scalar1=M_TH,
                                    scalar2=None, op0=mybir.AluOpType.is_ge)
                                nc.vector.tensor_copy(out=m16_sb, in_=m_sb)
                            nc.sync.dma_start(out=m_d[t, osl, :], in_=m16_sb)
                            nc.sync.dma_start(out=s_d[t, osl, :], in_=s_sb)
                            if t < T - 1:  # d is dead after the last step
                                u_sb = upool.tile([128, B], F32, tag="u")
                                nc.vector.tensor_scalar(
                                    out=u_sb, in0=m_sb, scalar1=M_TH,
                                    scalar2=ALPHA,
                                    op0=mybir.AluOpType.is_lt,
                                    op1=mybir.AluOpType.mult)
                                nc.vector.tensor_mul(out=u_sb, in0=m_sb,
                                                     in1=u_sb)
                                nc.scalar.add(d_t[ob], u_sb,
                                              bcol_t[:, ob:ob + 1])

                    if p == 0:
                        for kt in range(KT):
                            for ob in range(OB):
                                mm_hi(ob, kt)
                        for ob in range(OB):
                            drain(ob)
                    else:
                        for ob in range(OB):
                            for kt in range(KT):
                                mm_hi(ob, kt)
                            drain(ob)

            if reps == 1:
                body(xh0, warm=True)
            elif os.environ.get("BMU_UNROLL") == "1":
                body(xh0, warm=True)
                for _ in range(reps - 1):
                    body()
            else:
                body(xh0, warm=True)
                with tc.For_i(0, reps - 1, 1):
                    body()

    nc.compile()
    return nc


def _get_nc():
    if "nc" not in _cache:
        _cache["nc"] = _build_kernel()
    return _cache["nc"]


def _prepare_in_maps(x: np.ndarray, W: np.ndarray, b: np.ndarray):
    xT = np.ascontiguousarray(x.transpose(0, 2, 1))  # [T, D_in, B]

    def ptile16(a):  # [T, D, B] -> [NPASS, 128, KT*2*B], [kt][ti][b]
        return np.ascontiguousarray(
            a.reshape(NPASS, 2, KT, 128, B).transpose(0, 3, 2, 1, 4)
            .reshape(NPASS, 128, KT * 2 * B))

    def wtile16(a):  # [D, O] -> [128, KT*O]
        o = a.shape[1]
        return np.ascontiguousarray(
            a.reshape(KT, 128, o).transpose(1, 0, 2).reshape(128, KT * o))

    xh16_t = ptile16(xT.astype(NP_F16))

    in_maps = []
    for c in range(N_CORES):
        sl = slice(c * O_SHARD, (c + 1) * O_SHARD)
        Wt = np.ascontiguousarray(W[sl, :].T)  # [D, O]
        bcol = np.ascontiguousarray(
            b[sl].astype(np.float32).reshape(OB, 128).T)  # [128, OB]
        in_maps.append({
            "wh16": wtile16(Wt.astype(NP_F16)),
            "xh16": xh16_t,
            "bcol": bcol,
        })
    return in_maps


def kernel(x: np.ndarray, W: np.ndarray, b: np.ndarray):
    x = np.asarray(x, dtype=np.float32)
    W = np.asarray(W, dtype=np.float32)
    b = np.asarray(b, dtype=np.float32)
    nc = _get_nc()
    in_maps = _prepare_in_maps(x, W, b)
    res = run_bass_kernel_spmd(nc, in_maps, core_ids=list(range(N_CORES)))
    m = np.empty((T, B, D), dtype=np.float32)
    s = np.empty((T, B, D), dtype=np.float32)
    for c in range(N_CORES):
        sl = slice(c * O_SHARD, (c + 1) * O_SHARD)
        m[:, :, sl] = res.results[c]["m_out"].astype(np.float32) \
            .transpose(0, 2, 1)
        s[:, :, sl] = res.results[c]["s_out"].astype(np.float32) \
            .transpose(0, 2, 1)
    return (m, s)


# revision 12
# speedup vs baseline: 1.6408x; 1.6408x over previous
"""Trainium2 Bass kernel for nn_BoundMemUpdate (spiking membrane update).

Computes, for x:[T,B,D], W:[D,D], b:[D]:
    mm[t] = x[t] @ W.T + b
    m[t] = mm[t] + m[t-1] * (1 - s[t-1]) * 0.5
    s[t] = (m[t] >= 1.0)
Returns (m, s), each [T, B, D] float32.

Sharding: output-dim (D_out) sharded 8 ways across cores (512 each);
x replicated, W/b sharded by rows. The recurrence is per-neuron
elementwise, so no cross-core communication is needed.

Matmul: single-term fp16. x and W are rounded to fp16; the PE forms
exact fp16 products with fp32 accumulation, so the only error is the
input rounding (~2^-11 relative per operand). On the fixed seed-0
problem instance this measures 6.5e-3 concatenated relative error
(506 spike flips of 8.4M) against the fp32 reference -- 3x inside
the 2e-2 gate. W is the PE-stationary operand and x the moving one,
giving output tiles [o_part, (t,b)] and 512 matmuls of 518 cycles
per core (~111 us of PE streaming at 2.4 GHz).

Schedule: 4 passes over t-pairs, 4 o-block PSUM chains per pass with
K=4096 contraction, double-generation PSUM banks (8 total) so pass
boundaries never wait on drains. The vector engine fuses the drain
with the temporal recurrence; the bias enters through the scalar
engine's per-partition bias port (d-state starts at b and is
re-biased every step). x is stored pass-interleaved in DRAM
([pass][128][kt][t][b]) so every DMA is contiguous on both sides;
pass 0 streams k-outer so the PE consumes (W, x) chunks as they
land, and a short warmup matmul chain ramps the PE clock during the
cold 8 MB load. Outputs go out as m:fp16 / s:fp8 in [t, o, b]
layout; the host widens and transposes during the final gather.
"""
import os
import numpy as np

import concourse.bass as bass
import concourse.mybir as mybir
from concourse import bacc
from concourse.tile import TileContext
from concourse.bass_utils import run_bass_kernel_spmd

T, B, D = 8, 256, 4096
N_CORES = 8
O_SHARD = D // N_CORES   # 512
KT = D // 128            # 32 fp16 k-tiles
OB = O_SHARD // 128      # 4 output blocks
NPASS = T // 2           # 4 t-pair passes
ALPHA = 0.5
M_TH = 1.0
WARM_MM = int(os.environ.get("BMU_WARM", "32"))
PSB = int(os.environ.get("BMU_PSB", "2"))       # psum pool bufs
XCH = int(os.environ.get("BMU_XCH", "8"))       # x DMA chunks per pass
OFFLOAD = os.environ.get("BMU_OFFLOAD", "0") == "1"  # s8->gpsimd, m16->scalar

F16 = mybir.dt.float16
F8 = mybir.dt.float8e4
F32 = mybir.dt.float32
NP_F16 = np.float16

_cache = {}


def _build_kernel(reps: int = 1):
    nc = bacc.Bacc("TRN2", target_bir_lowering=False, debug=False,
                   num_devices=N_CORES)

    NF16 = KT * 2 * B        # per-pass x free size (16384)

    wh16_d = nc.dram_tensor("wh16", [128, KT * O_SHARD], F16,
                            kind="ExternalInput").ap()
    xh16_d = nc.dram_tensor("xh16", [NPASS, 128, NF16], F16,
                            kind="ExternalInput").ap()
    bcol_d = nc.dram_tensor("bcol", [128, OB], F32,
                            kind="ExternalInput").ap()
    m_d = nc.dram_tensor("m_out", [T, O_SHARD, B], F16,
                         kind="ExternalOutput").ap()
    s_d = nc.dram_tensor("s_out", [T, O_SHARD, B], F8,
                         kind="ExternalOutput").ap()

    with TileContext(nc) as tc:
        with tc.tile_pool(name="wpool", bufs=1) as wpool, \
             tc.tile_pool(name="xhpool", bufs=2) as xhpool, \
             tc.tile_pool(name="cpool", bufs=1) as cpool, \
             tc.tile_pool(name="mpool", bufs=4) as mpool, \
             tc.tile_pool(name="opool", bufs=4) as opool, \
             tc.tile_pool(name="spool", bufs=4) as spool, \
             tc.tile_pool(name="upool", bufs=3) as upool, \
             tc.tile_pool(name="psum", bufs=PSB, space="PSUM") as psum_pool:

            whs = wpool.tile([128, KT * O_SHARD], F16, name="whs")
            bcol_t = cpool.tile([128, OB], F32)
            d_t = [cpool.tile([128, B], F32, name=f"d{ob}") for ob in range(OB)]
            warm_t = cpool.tile([128, 2 * B], F16, name="warm")
            dump_t = cpool.tile([128, 16], F32, name="dump")

            xsz = NF16 // XCH
            WCH = 8
            wsz = KT * O_SHARD // WCH

            # pass-0 load, interleaved in PE consumption order (k-outer
            # pass 0 walks kt 0..31 across o-blocks, so pair (wh16, xh16)
            # chunks by kt range).
            xh0 = xhpool.tile([128, NF16], F16, tag="xh")
            nc.sync.dma_start(out=bcol_t, in_=bcol_d)
            for c in range(XCH):
                wsl = slice(c * wsz, (c + 1) * wsz)
                nc.sync.dma_start(out=whs[:, wsl], in_=wh16_d[:, wsl])
                xsl = slice(c * xsz, (c + 1) * xsz)
                nc.sync.dma_start(out=xh0[:, xsl], in_=xh16_d[0][:, xsl])

            wh_k = whs.rearrange("p (kt o) -> p kt o", kt=KT)

            def warmup():
                if WARM_MM <= 0:
                    return
                nc.vector.memset(warm_t, 0.0)
                wp = psum_pool.tile([128, 2 * B], F32, tag="hi3",
                                    name="warmpsum")
                for i in range(WARM_MM):
                    nc.tensor.matmul(wp, warm_t[:, :128], warm_t,
                                     start=(i == 0), stop=(i == WARM_MM - 1))
                nc.vector.tensor_copy(out=dump_t, in_=wp[:, :16])

            def body(first=None, warm=False):
                for ob in range(OB):
                    nc.vector.memset(d_t[ob], 0.0)
                    nc.scalar.add(d_t[ob], d_t[ob], bcol_t[:, ob:ob + 1])
                if warm:
                    warmup()
                for p in range(NPASS):
                    if p == 0 and first is not None:
                        xh = first
                    else:
                        xh = xhpool.tile([128, NF16], F16, tag="xh")
                        for c in range(XCH):
                            xsl = slice(c * xsz, (c + 1) * xsz)
                            nc.sync.dma_start(out=xh[:, xsl],
                                              in_=xh16_d[p][:, xsl])

                    xh_kv = xh.rearrange("p (kt n) -> p kt n", kt=KT)

                    hi_t = [psum_pool.tile([128, 2 * B], F32, tag=f"hi{ob}",
                                           name=f"hi{p}_{ob}")
                            for ob in range(OB)]

                    def mm_hi(ob, kt):
                        osl = slice(ob * 128, (ob + 1) * 128)
                        nc.tensor.matmul(hi_t[ob], wh_k[:, kt, osl],
                                         xh_kv[:, kt, :],
                                         start=(kt == 0), stop=(kt == KT - 1))

                    def drain(ob):
                        osl = slice(ob * 128, (ob + 1) * 128)
                        for ti in range(2):
                            t = 2 * p + ti
                            bsl = slice(ti * B, (ti + 1) * B)
                            m_sb = mpool.tile([128, B], F32, tag="m")
                            nc.vector.tensor_add(out=m_sb,
                                                 in0=hi_t[ob][:, bsl],
                                                 in1=d_t[ob])
                            s_sb = spool.tile([128, B], F8, tag="s")
                            m16_sb = opool.tile([128, B], F16, tag="m16")
                            if OFFLOAD:
                                nc.gpsimd.tensor_scalar(
                                    out=s_sb, in0=m_sb, scalar1=M_TH,
                                    scalar2=None, op0=mybir.AluOpType.is_ge)
                                nc.scalar.copy(m16_sb, m_sb)
                            else:
                                nc.vector.tensor_scalar(
                                    out=s_sb, in0=m_sb, scalar1=M_TH,
                                    scalar2=None, op0=mybir.AluOpType.is_ge)
                                nc.vector.tensor_copy(out=m16_sb, in_=m_sb)
                            nc.sync.dma_start(out=m_d[t, osl, :], in_=m16_sb)
                            nc.sync.dma_start(out=s_d[t, osl, :], in_=s_sb)
                            if t < T - 1:  # d is dead after the last step
                                u_sb = upool.tile([128, B], F32, tag="u")
                                nc.vector.tensor_scalar(
                                    out=u_sb, in0=m_sb, scalar1=M_TH,
                                    scalar2=ALPHA,
                                    op0=mybir.AluOpType.is_lt,
                                    op1=mybir.AluOpType.mult)
                                nc.vector.tensor_mul(out=u_sb, in0=m_sb,
                                                     in1=u_sb)
                                nc.scalar.add(d_t[ob], u_sb,
                                              bcol_t[:, ob:ob + 1])

                    if p == 0:
                        for kt in range(KT):
                            for ob in range(OB):
                                mm_hi(ob, kt)
                        for ob in range(OB):
                            drain(ob)
                    else:
                        for ob in range(OB):
                            for kt in range(KT):
                                mm_hi(ob, kt)
                            drain(ob)

            if reps == 1:
                body(xh0, warm=True)
            elif os.environ.get("BMU_UNROLL") == "1":
                body(xh0, warm=True)
                for _ in range(reps - 1):
                    body()
            else:
                body(xh0, warm=True)
                with tc.For_i(0, reps - 1, 1):
                    body()

    nc.compile()
    return nc


def _get_nc():
    if "nc" not in _cache:
        _cache["nc"] = _build_kernel()
    return _cache["nc"]


def _prepare_in_maps(x: np.ndarray, W: np.ndarray, b: np.ndarray):
    xT = np.ascontiguousarray(x.transpose(0, 2, 1))  # [T, D_in, B]

    def ptile16(a):  # [T, D, B] -> [NPASS, 128, KT*2*B], [kt][ti][b]
        return np.ascontiguousarray(
            a.reshape(NPASS, 2, KT, 128, B).transpose(0, 3, 2, 1, 4)
            .reshape(NPASS, 128, KT * 2 * B))

    def wtile16(a):  # [D, O] -> [128, KT*O]
        o = a.shape[1]
        return np.ascontiguousarray(
            a.reshape(KT, 128, o).transpose(1, 0, 2).reshape(128, KT * o))

    xh16_t = ptile16(xT.astype(NP_F16))

    in_maps = []
    for c in range(N_CORES):
        sl = slice(c * O_SHARD, (c + 1) * O_SHARD)
        Wt = np.ascontiguousarray(W[sl, :].T)  # [D, O]
        bcol = np.ascontiguousarray(
            b[sl].astype(np.float32).reshape(OB, 128).T)  # [128, OB]
        in_maps.append({
            "wh16": wtile16(Wt.astype(NP_F16)),
            "xh16": xh16_t,
            "bcol": bcol,
        })
    return in_maps


def kernel(x: np.ndarray, W: np.ndarray, b: np.ndarray):
    x = np.asarray(x, dtype=np.float32)
    W = np.asarray(W, dtype=np.float32)
    b = np.asarray(b, dtype=np.float32)
    nc = _get_nc()
    in_maps = _prepare_in_maps(x, W, b)
    res = run_bass_kernel_spmd(nc, in_maps, core_ids=list(range(N_CORES)))
    m = np.empty((T, B, D), dtype=np.float32)
    s = np.empty((T, B, D), dtype=np.float32)
    for c in range(N_CORES):
        sl = slice(c * O_SHARD, (c + 1) * O_SHARD)
        m[:, :, sl] = res.results[c]["m_out"].astype(np.float32) \
            .transpose(0, 2, 1)
        s[:, :, sl] = res.results[c]["s_out"].astype(np.float32) \
            .transpose(0, 2, 1)
    return (m, s)


# revision 13
# speedup vs baseline: 1.6686x; 1.0170x over previous
"""Trainium2 Bass kernel for nn_BoundMemUpdate (spiking membrane update).

Computes, for x:[T,B,D], W:[D,D], b:[D]:
    mm[t] = x[t] @ W.T + b
    m[t] = mm[t] + m[t-1] * (1 - s[t-1]) * 0.5
    s[t] = (m[t] >= 1.0)
Returns (m, s), each [T, B, D] float32.

Sharding: output-dim (D_out) sharded 8 ways across cores (512 each);
x replicated, W/b sharded by rows. The recurrence is per-neuron
elementwise, so no cross-core communication is needed.

Matmul: single-term fp16. x and W are rounded to fp16; the PE forms
exact fp16 products with fp32 accumulation, so the only error is the
input rounding (~2^-11 relative per operand). On the fixed seed-0
problem instance this measures 6.5e-3 concatenated relative error
(506 spike flips of 8.4M) against the fp32 reference -- 3x inside
the 2e-2 gate. W is the PE-stationary operand and x the moving one,
giving output tiles [o_part, (t,b)] and 512 matmuls of 518 cycles
per core (~111 us of PE streaming at 2.4 GHz).

Schedule: 4 passes over t-pairs, 4 o-block PSUM chains per pass with
K=4096 contraction, double-generation PSUM banks (8 total) so pass
boundaries never wait on drains. The vector engine fuses the drain
with the temporal recurrence; the bias enters through the scalar
engine's per-partition bias port (d-state starts at b and is
re-biased every step). x is stored pass-interleaved in DRAM
([pass][128][kt][t][b]) so every DMA is contiguous on both sides;
pass 0 streams k-outer so the PE consumes (W, x) chunks as they
land, and a short warmup matmul chain ramps the PE clock during the
cold 8 MB load. Outputs go out as m:fp16 / s:fp8 in [t, o, b]
layout; the host widens and transposes during the final gather.
"""
import os
import numpy as np

import concourse.bass as bass
import concourse.mybir as mybir
from concourse import bacc
from concourse.tile import TileContext
from concourse.bass_utils import run_bass_kernel_spmd

T, B, D = 8, 256, 4096
N_CORES = 8
O_SHARD = D // N_CORES   # 512
KT = D // 128            # 32 fp16 k-tiles
OB = O_SHARD // 128      # 4 output blocks
NPASS = T // 2           # 4 t-pair passes
ALPHA = 0.5
M_TH = 1.0
WARM_MM = int(os.environ.get("BMU_WARM", "32"))
PSB = int(os.environ.get("BMU_PSB", "1"))       # psum pool bufs
XCH = int(os.environ.get("BMU_XCH", "8"))       # x DMA chunks per pass
OFFLOAD = os.environ.get("BMU_OFFLOAD", "0") == "1"  # s8->gpsimd, m16->scalar

F16 = mybir.dt.float16
F8 = mybir.dt.float8e4
F32 = mybir.dt.float32
NP_F16 = np.float16

_cache = {}


def _build_kernel(reps: int = 1):
    nc = bacc.Bacc("TRN2", target_bir_lowering=False, debug=False,
                   num_devices=N_CORES)

    NF16 = KT * 2 * B        # per-pass x free size (16384)

    wh16_d = nc.dram_tensor("wh16", [128, KT * O_SHARD], F16,
                            kind="ExternalInput").ap()
    xh16_d = nc.dram_tensor("xh16", [NPASS, 128, NF16], F16,
                            kind="ExternalInput").ap()
    bcol_d = nc.dram_tensor("bcol", [128, OB], F32,
                            kind="ExternalInput").ap()
    m_d = nc.dram_tensor("m_out", [T, O_SHARD, B], F16,
                         kind="ExternalOutput").ap()
    s_d = nc.dram_tensor("s_out", [T, O_SHARD, B], F8,
                         kind="ExternalOutput").ap()

    with TileContext(nc) as tc:
        with tc.tile_pool(name="wpool", bufs=1) as wpool, \
             tc.tile_pool(name="xhpool", bufs=2) as xhpool, \
             tc.tile_pool(name="cpool", bufs=1) as cpool, \
             tc.tile_pool(name="mpool", bufs=4) as mpool, \
             tc.tile_pool(name="opool", bufs=4) as opool, \
             tc.tile_pool(name="spool", bufs=4) as spool, \
             tc.tile_pool(name="upool", bufs=3) as upool, \
             tc.tile_pool(name="psum", bufs=PSB, space="PSUM") as psum_pool:

            whs = wpool.tile([128, KT * O_SHARD], F16, name="whs")
            bcol_t = cpool.tile([128, OB], F32)
            d_t = [cpool.tile([128, B], F32, name=f"d{ob}") for ob in range(OB)]
            warm_t = cpool.tile([128, 2 * B], F16, name="warm")
            dump_t = cpool.tile([128, 16], F32, name="dump")

            xsz = NF16 // XCH
            WCH = 8
            wsz = KT * O_SHARD // WCH

            # pass-0 load, interleaved in PE consumption order (k-outer
            # pass 0 walks kt 0..31 across o-blocks, so pair (wh16, xh16)
            # chunks by kt range).
            xh0 = xhpool.tile([128, NF16], F16, tag="xh")
            nc.sync.dma_start(out=bcol_t, in_=bcol_d)
            for c in range(XCH):
                wsl = slice(c * wsz, (c + 1) * wsz)
                nc.sync.dma_start(out=whs[:, wsl], in_=wh16_d[:, wsl])
                xsl = slice(c * xsz, (c + 1) * xsz)
                nc.sync.dma_start(out=xh0[:, xsl], in_=xh16_d[0][:, xsl])

            wh_k = whs.rearrange("p (kt o) -> p kt o", kt=KT)

            def warmup():
                if WARM_MM <= 0:
                    return
                nc.gpsimd.memset(warm_t, 0.0)
                wp = psum_pool.tile([128, 2 * B], F32, tag="hi3",
                                    name="warmpsum")
                for i in range(WARM_MM):
                    nc.tensor.matmul(wp, warm_t[:, :128], warm_t,
                                     start=(i == 0), stop=(i == WARM_MM - 1))
                nc.vector.tensor_copy(out=dump_t, in_=wp[:, :16])

            def body(first=None, warm=False):
                for ob in range(OB):
                    nc.vector.memset(d_t[ob], 0.0)
                    nc.scalar.add(d_t[ob], d_t[ob], bcol_t[:, ob:ob + 1])
                if warm:
                    warmup()
                for p in range(NPASS):
                    if p == 0 and first is not None:
                        xh = first
                    else:
                        xh = xhpool.tile([128, NF16], F16, tag="xh")
                        for c in range(XCH):
                            xsl = slice(c * xsz, (c + 1) * xsz)
                            nc.sync.dma_start(out=xh[:, xsl],
                                              in_=xh16_d[p][:, xsl])

                    xh_kv = xh.rearrange("p (kt n) -> p kt n", kt=KT)

                    hi_t = [psum_pool.tile([128, 2 * B], F32, tag=f"hi{ob}",
                                           name=f"hi{p}_{ob}")
                            for ob in range(OB)]

                    def mm_hi(ob, kt):
                        osl = slice(ob * 128, (ob + 1) * 128)
                        nc.tensor.matmul(hi_t[ob], wh_k[:, kt, osl],
                                         xh_kv[:, kt, :],
                                         start=(kt == 0), stop=(kt == KT - 1))

                    def drain(ob):
                        osl = slice(ob * 128, (ob + 1) * 128)
                        for ti in range(2):
                            t = 2 * p + ti
                            bsl = slice(ti * B, (ti + 1) * B)
                            m_sb = mpool.tile([128, B], F32, tag="m")
                            nc.vector.tensor_add(out=m_sb,
                                                 in0=hi_t[ob][:, bsl],
                                                 in1=d_t[ob])
                            s_sb = spool.tile([128, B], F8, tag="s")
                            m16_sb = opool.tile([128, B], F16, tag="m16")
                            if OFFLOAD:
                                nc.gpsimd.tensor_scalar(
                                    out=s_sb, in0=m_sb, scalar1=M_TH,
                                    scalar2=None, op0=mybir.AluOpType.is_ge)
                                nc.scalar.copy(m16_sb, m_sb)
                            else:
                                nc.vector.tensor_scalar(
                                    out=s_sb, in0=m_sb, scalar1=M_TH,
                                    scalar2=None, op0=mybir.AluOpType.is_ge)
                                nc.vector.tensor_copy(out=m16_sb, in_=m_sb)
                            nc.sync.dma_start(out=m_d[t, osl, :], in_=m16_sb)
                            nc.sync.dma_start(out=s_d[t, osl, :], in_=s_sb)
                            if t < T - 1:  # d is dead after the last step
                                u_sb = upool.tile([128, B], F32, tag="u")
                                nc.vector.tensor_scalar(
                                    out=u_sb, in0=m_sb, scalar1=M_TH,
                                    scalar2=ALPHA,
                                    op0=mybir.AluOpType.is_lt,
                                    op1=mybir.AluOpType.mult)
                                nc.vector.tensor_mul(out=u_sb, in0=m_sb,
                                                     in1=u_sb)
                                nc.scalar.add(d_t[ob], u_sb,
                                              bcol_t[:, ob:ob + 1])

                    if p == 0:
                        # k-outer while the cold DMA streams in, then
                        # ob-sequential so the chains finish staggered
                        # and drains overlap the tail of the pass.
                        KSPLIT = 24
                        for kt in range(KSPLIT):
                            for ob in range(OB):
                                mm_hi(ob, kt)
                        for ob in range(OB):
                            for kt in range(KSPLIT, KT):
                                mm_hi(ob, kt)
                            drain(ob)
                    else:
                        for ob in range(OB):
                            for kt in range(KT):
                                mm_hi(ob, kt)
                            drain(ob)

            if reps == 1:
                body(xh0, warm=True)
            elif os.environ.get("BMU_UNROLL") == "1":
                body(xh0, warm=True)
                for _ in range(reps - 1):
                    body()
            else:
                body(xh0, warm=True)
                with tc.For_i(0, reps - 1, 1):
                    body()

    nc.compile()
    return nc


def _get_nc():
    if "nc" not in _cache:
        _cache["nc"] = _build_kernel()
    return _cache["nc"]


def _prepare_in_maps(x: np.ndarray, W: np.ndarray, b: np.ndarray):
    xT = np.ascontiguousarray(x.transpose(0, 2, 1))  # [T, D_in, B]

    def ptile16(a):  # [T, D, B] -> [NPASS, 128, KT*2*B], [kt][ti][b]
        return np.ascontiguousarray(
            a.reshape(NPASS, 2, KT, 128, B).transpose(0, 3, 2, 1, 4)
            .reshape(NPASS, 128, KT * 2 * B))

    def wtile16(a):  # [D, O] -> [128, KT*O]
        o = a.shape[1]
        return np.ascontiguousarray(
            a.reshape(KT, 128, o).transpose(1, 0, 2).reshape(128, KT * o))

    xh16_t = ptile16(xT.astype(NP_F16))

    in_maps = []
    for c in range(N_CORES):
        sl = slice(c * O_SHARD, (c + 1) * O_SHARD)
        Wt = np.ascontiguousarray(W[sl, :].T)  # [D, O]
        bcol = np.ascontiguousarray(
            b[sl].astype(np.float32).reshape(OB, 128).T)  # [128, OB]
        in_maps.append({
            "wh16": wtile16(Wt.astype(NP_F16)),
            "xh16": xh16_t,
            "bcol": bcol,
        })
    return in_maps


def kernel(x: np.ndarray, W: np.ndarray, b: np.ndarray):
    x = np.asarray(x, dtype=np.float32)
    W = np.asarray(W, dtype=np.float32)
    b = np.asarray(b, dtype=np.float32)
    nc = _get_nc()
    in_maps = _prepare_in_maps(x, W, b)
    res = run_bass_kernel_spmd(nc, in_maps, core_ids=list(range(N_CORES)))
    m = np.empty((T, B, D), dtype=np.float32)
    s = np.empty((T, B, D), dtype=np.float32)
    for c in range(N_CORES):
        sl = slice(c * O_SHARD, (c + 1) * O_SHARD)
        m[:, :, sl] = res.results[c]["m_out"].astype(np.float32) \
            .transpose(0, 2, 1)
        s[:, :, sl] = res.results[c]["s_out"].astype(np.float32) \
            .transpose(0, 2, 1)
    return (m, s)


# revision 14
# speedup vs baseline: 1.6696x; 1.0006x over previous
"""Trainium2 Bass kernel for nn_BoundMemUpdate (spiking membrane update).

Computes, for x:[T,B,D], W:[D,D], b:[D]:
    mm[t] = x[t] @ W.T + b
    m[t] = mm[t] + m[t-1] * (1 - s[t-1]) * 0.5
    s[t] = (m[t] >= 1.0)
Returns (m, s), each [T, B, D] float32.

Sharding: output-dim (D_out) sharded 8 ways across cores (512 each);
x replicated, W/b sharded by rows. The recurrence is per-neuron
elementwise, so no cross-core communication is needed.

Matmul: single-term fp16. x and W are rounded to fp16; the PE forms
exact fp16 products with fp32 accumulation, so the only error is the
input rounding (~2^-11 relative per operand). On the fixed seed-0
problem instance this measures 6.5e-3 concatenated relative error
(506 spike flips of 8.4M) against the fp32 reference -- 3x inside
the 2e-2 gate. W is the PE-stationary operand and x the moving one,
giving output tiles [o_part, (t,b)] and 512 matmuls of 518 cycles
per core (~111 us of PE streaming at 2.4 GHz).

Schedule: 4 passes over t-pairs, 4 o-block PSUM chains per pass with
K=4096 contraction, double-generation PSUM banks (8 total) so pass
boundaries never wait on drains. The vector engine fuses the drain
with the temporal recurrence; the bias enters through the scalar
engine's per-partition bias port (d-state starts at b and is
re-biased every step). x is stored pass-interleaved in DRAM
([pass][128][kt][t][b]) so every DMA is contiguous on both sides;
pass 0 streams k-outer so the PE consumes (W, x) chunks as they
land, and a short warmup matmul chain ramps the PE clock during the
cold 8 MB load. Outputs go out as m:fp16 / s:fp8 in [t, o, b]
layout; the host widens and transposes during the final gather.
"""
import os
import numpy as np

import concourse.bass as bass
import concourse.mybir as mybir
from concourse import bacc
from concourse.tile import TileContext
from concourse.bass_utils import run_bass_kernel_spmd

T, B, D = 8, 256, 4096
N_CORES = 8
O_SHARD = D // N_CORES   # 512
KT = D // 128            # 32 fp16 k-tiles
OB = O_SHARD // 128      # 4 output blocks
NPASS = T // 2           # 4 t-pair passes
ALPHA = 0.5
M_TH = 1.0
WARM_MM = int(os.environ.get("BMU_WARM", "32"))
PSB = int(os.environ.get("BMU_PSB", "1"))       # psum pool bufs
XCH = int(os.environ.get("BMU_XCH", "8"))       # x DMA chunks per pass
OFFLOAD = os.environ.get("BMU_OFFLOAD", "0") == "1"  # s8->gpsimd, m16->scalar

F16 = mybir.dt.float16
F8 = mybir.dt.float8e4
F32 = mybir.dt.float32
NP_F16 = np.float16

_cache = {}


def _build_kernel(reps: int = 1):
    nc = bacc.Bacc("TRN2", target_bir_lowering=False, debug=False,
                   num_devices=N_CORES)

    NF16 = KT * 2 * B        # per-pass x free size (16384)

    wh16_d = nc.dram_tensor("wh16", [128, KT * O_SHARD], F16,
                            kind="ExternalInput").ap()
    xh16_d = nc.dram_tensor("xh16", [NPASS, 128, NF16], F16,
                            kind="ExternalInput").ap()
    bcol_d = nc.dram_tensor("bcol", [128, OB], F32,
                            kind="ExternalInput").ap()
    m_d = nc.dram_tensor("m_out", [T, O_SHARD, B], F16,
                         kind="ExternalOutput").ap()
    s_d = nc.dram_tensor("s_out", [T, O_SHARD, B], F8,
                         kind="ExternalOutput").ap()

    with TileContext(nc) as tc:
        with tc.tile_pool(name="wpool", bufs=1) as wpool, \
             tc.tile_pool(name="xhpool", bufs=2) as xhpool, \
             tc.tile_pool(name="cpool", bufs=1) as cpool, \
             tc.tile_pool(name="mpool", bufs=4) as mpool, \
             tc.tile_pool(name="opool", bufs=4) as opool, \
             tc.tile_pool(name="spool", bufs=4) as spool, \
             tc.tile_pool(name="upool", bufs=3) as upool, \
             tc.tile_pool(name="psum", bufs=PSB, space="PSUM") as psum_pool:

            whs = wpool.tile([128, KT * O_SHARD], F16, name="whs")
            bcol_t = cpool.tile([128, OB], F32)
            d_t = [cpool.tile([128, B], F32, name=f"d{ob}") for ob in range(OB)]
            warm_t = cpool.tile([128, 2 * B], F16, name="warm")
            dump_t = cpool.tile([128, 16], F32, name="dump")

            xsz = NF16 // XCH
            WCH = 8
            wsz = KT * O_SHARD // WCH

            # pass-0 load, interleaved in PE consumption order (k-outer
            # pass 0 walks kt 0..31 across o-blocks, so pair (wh16, xh16)
            # chunks by kt range).
            xh0 = xhpool.tile([128, NF16], F16, tag="xh")
            nc.sync.dma_start(out=bcol_t, in_=bcol_d)
            for c in range(XCH):
                wsl = slice(c * wsz, (c + 1) * wsz)
                nc.sync.dma_start(out=whs[:, wsl], in_=wh16_d[:, wsl])
                xsl = slice(c * xsz, (c + 1) * xsz)
                nc.sync.dma_start(out=xh0[:, xsl], in_=xh16_d[0][:, xsl])

            wh_k = whs.rearrange("p (kt o) -> p kt o", kt=KT)

            def warmup():
                if WARM_MM <= 0:
                    return
                nc.gpsimd.memset(warm_t, 0.0)
                wp = psum_pool.tile([128, 2 * B], F32, tag="hi3",
                                    name="warmpsum")
                for i in range(WARM_MM):
                    nc.tensor.matmul(wp, warm_t[:, :128], warm_t,
                                     start=(i == 0), stop=(i == WARM_MM - 1))
                nc.vector.tensor_copy(out=dump_t, in_=wp[:, :16])

            def body(first=None, warm=False):
                for ob in range(OB):
                    nc.vector.memset(d_t[ob], 0.0)
                    nc.scalar.add(d_t[ob], d_t[ob], bcol_t[:, ob:ob + 1])
                if warm:
                    warmup()
                for p in range(NPASS):
                    if p == 0 and first is not None:
                        xh = first
                    else:
                        xh = xhpool.tile([128, NF16], F16, tag="xh")
                        for c in range(XCH):
                            xsl = slice(c * xsz, (c + 1) * xsz)
                            nc.sync.dma_start(out=xh[:, xsl],
                                              in_=xh16_d[p][:, xsl])

                    xh_kv = xh.rearrange("p (kt n) -> p kt n", kt=KT)

                    hi_t = [psum_pool.tile([128, 2 * B], F32, tag=f"hi{ob}",
                                           name=f"hi{p}_{ob}")
                            for ob in range(OB)]

                    def mm_hi(ob, kt):
                        osl = slice(ob * 128, (ob + 1) * 128)
                        nc.tensor.matmul(hi_t[ob], wh_k[:, kt, osl],
                                         xh_kv[:, kt, :],
                                         start=(kt == 0), stop=(kt == KT - 1))

                    def drain(ob):
                        osl = slice(ob * 128, (ob + 1) * 128)
                        for ti in range(2):
                            t = 2 * p + ti
                            bsl = slice(ti * B, (ti + 1) * B)
                            m_sb = mpool.tile([128, B], F32, tag="m")
                            nc.vector.tensor_add(out=m_sb,
                                                 in0=hi_t[ob][:, bsl],
                                                 in1=d_t[ob])
                            s_sb = spool.tile([128, B], F8, tag="s")
                            m16_sb = opool.tile([128, B], F16, tag="m16")
                            if OFFLOAD:
                                nc.gpsimd.tensor_scalar(
                                    out=s_sb, in0=m_sb, scalar1=M_TH,
                                    scalar2=None, op0=mybir.AluOpType.is_ge)
                                nc.scalar.copy(m16_sb, m_sb)
                            else:
                                nc.vector.tensor_scalar(
                                    out=s_sb, in0=m_sb, scalar1=M_TH,
                                    scalar2=None, op0=mybir.AluOpType.is_ge)
                                nc.vector.tensor_copy(out=m16_sb, in_=m_sb)
                            nc.sync.dma_start(out=m_d[t, osl, :], in_=m16_sb)
                            nc.sync.dma_start(out=s_d[t, osl, :], in_=s_sb)
                            if t < T - 1:  # d is dead after the last step
                                u_sb = upool.tile([128, B], F32, tag="u")
                                nc.vector.tensor_scalar(
                                    out=u_sb, in0=m_sb, scalar1=M_TH,
                                    scalar2=ALPHA,
                                    op0=mybir.AluOpType.is_lt,
                                    op1=mybir.AluOpType.mult)
                                nc.vector.tensor_mul(out=u_sb, in0=m_sb,
                                                     in1=u_sb)
                                nc.scalar.add(d_t[ob], u_sb,
                                              bcol_t[:, ob:ob + 1])

                    if p == 0:
                        # k-outer while the cold DMA streams in, then
                        # ob-sequential so the chains finish staggered
                        # and drains overlap the tail of the pass.
                        KSPLIT = 24
                        for kt in range(KSPLIT):
                            for ob in range(OB):
                                mm_hi(ob, kt)
                        for ob in range(OB):
                            for kt in range(KSPLIT, KT):
                                mm_hi(ob, kt)
                            drain(ob)
                    else:
                        for ob in range(OB):
                            for kt in range(KT):
                                mm_hi(ob, kt)
                            drain(ob)

            if reps == 1:
                body(xh0, warm=True)
            elif os.environ.get("BMU_UNROLL") == "1":
                body(xh0, warm=True)
                for _ in range(reps - 1):
                    body()
            else:
                body(xh0, warm=True)
                with tc.For_i(0, reps - 1, 1):
                    body()

    nc.compile()
    return nc


def _get_nc():
    if "nc" not in _cache:
        _cache["nc"] = _build_kernel()
    return _cache["nc"]


def _prepare_in_maps(x: np.ndarray, W: np.ndarray, b: np.ndarray):
    xT = np.ascontiguousarray(x.transpose(0, 2, 1))  # [T, D_in, B]

    def ptile16(a):  # [T, D, B] -> [NPASS, 128, KT*2*B], [kt][ti][b]
        return np.ascontiguousarray(
            a.reshape(NPASS, 2, KT, 128, B).transpose(0, 3, 2, 1, 4)
            .reshape(NPASS, 128, KT * 2 * B))

    def wtile16(a):  # [D, O] -> [128, KT*O]
        o = a.shape[1]
        return np.ascontiguousarray(
            a.reshape(KT, 128, o).transpose(1, 0, 2).reshape(128, KT * o))

    xh16_t = ptile16(xT.astype(NP_F16))

    in_maps = []
    for c in range(N_CORES):
        sl = slice(c * O_SHARD, (c + 1) * O_SHARD)
        Wt = np.ascontiguousarray(W[sl, :].T)  # [D, O]
        bcol = np.ascontiguousarray(
            b[sl].astype(np.float32).reshape(OB, 128).T)  # [128, OB]
        in_maps.append({
            "wh16": wtile16(Wt.astype(NP_F16)),
            "xh16": xh16_t,
            "bcol": bcol,
        })
    return in_maps


def kernel(x: np.ndarray, W: np.ndarray, b: np.ndarray):
    x = np.asarray(x, dtype=np.float32)
    W = np.asarray(W, dtype=np.float32)
    b = np.asarray(b, dtype=np.float32)
    nc = _get_nc()
    in_maps = _prepare_in_maps(x, W, b)
    res = None
    for attempt in range(3):
        try:
            res = run_bass_kernel_spmd(nc, in_maps,
                                       core_ids=list(range(N_CORES)))
            break
        except Exception:
            # transient device errors (NRT INTERNAL/UNRECOVERABLE) clear
            # on retry; re-raise only if persistent
            if attempt == 2:
                raise
    m = np.empty((T, B, D), dtype=np.float32)
    s = np.empty((T, B, D), dtype=np.float32)
    for c in range(N_CORES):
        sl = slice(c * O_SHARD, (c + 1) * O_SHARD)
        m[:, :, sl] = res.results[c]["m_out"].astype(np.float32) \
            .transpose(0, 2, 1)
        s[:, :, sl] = res.results[c]["s_out"].astype(np.float32) \
            .transpose(0, 2, 1)
    return (m, s)


# revision 15
# speedup vs baseline: 1.6735x; 1.0023x over previous
"""Trainium2 Bass kernel for nn_BoundMemUpdate (spiking membrane update).

Computes, for x:[T,B,D], W:[D,D], b:[D]:
    mm[t] = x[t] @ W.T + b
    m[t] = mm[t] + m[t-1] * (1 - s[t-1]) * 0.5
    s[t] = (m[t] >= 1.0)
Returns (m, s), each [T, B, D] float32.

Sharding: output-dim (D_out) sharded 8 ways across cores (512 each);
x replicated, W/b sharded by rows. The recurrence is per-neuron
elementwise, so no cross-core communication is needed.

Matmul: single-term fp16. x and W are rounded to fp16; the PE forms
exact fp16 products with fp32 accumulation, so the only error is the
input rounding (~2^-11 relative per operand). On the fixed seed-0
problem instance this measures 6.5e-3 concatenated relative error
(506 spike flips of 8.4M) against the fp32 reference -- 3x inside
the 2e-2 gate. W is the PE-stationary operand and x the moving one,
giving output tiles [o_part, (t,b)] and 512 matmuls of 518 cycles
per core (~111 us of PE streaming at 2.4 GHz).

Schedule: 4 passes over t-pairs, 4 o-block PSUM chains per pass with
K=4096 contraction, double-generation PSUM banks (8 total) so pass
boundaries never wait on drains. The vector engine fuses the drain
with the temporal recurrence; the bias enters through the scalar
engine's per-partition bias port (d-state starts at b and is
re-biased every step). x is stored pass-interleaved in DRAM
([pass][128][kt][t][b]) so every DMA is contiguous on both sides;
pass 0 streams k-outer so the PE consumes (W, x) chunks as they
land, and a short warmup matmul chain ramps the PE clock during the
cold 8 MB load. Outputs go out as m:fp16 / s:fp8 in [t, o, b]
layout; the host widens and transposes during the final gather.
"""
import os
import numpy as np

import concourse.bass as bass
import concourse.mybir as mybir
from concourse import bacc
from concourse.tile import TileContext
from concourse.bass_utils import run_bass_kernel_spmd

T, B, D = 8, 256, 4096
N_CORES = 8
O_SHARD = D // N_CORES   # 512
KT = D // 128            # 32 fp16 k-tiles
OB = O_SHARD // 128      # 4 output blocks
NPASS = T // 2           # 4 t-pair passes
ALPHA = 0.5
M_TH = 1.0
WARM_MM = int(os.environ.get("BMU_WARM", "32"))
PSB = int(os.environ.get("BMU_PSB", "1"))       # psum pool bufs
XCH = int(os.environ.get("BMU_XCH", "8"))       # x DMA chunks per pass
WCH = int(os.environ.get("BMU_WCH", "8"))       # W DMA chunks
OFFLOAD = os.environ.get("BMU_OFFLOAD", "0") == "1"  # s8->gpsimd, m16->scalar

F16 = mybir.dt.float16
F8 = mybir.dt.float8e4
F32 = mybir.dt.float32
NP_F16 = np.float16

_cache = {}


def _build_kernel(reps: int = 1):
    nc = bacc.Bacc("TRN2", target_bir_lowering=False, debug=False,
                   num_devices=N_CORES)

    NF16 = KT * 2 * B        # per-pass x free size (16384)

    wh16_d = nc.dram_tensor("wh16", [128, KT * O_SHARD], F16,
                            kind="ExternalInput").ap()
    xh16_d = nc.dram_tensor("xh16", [NPASS, 128, NF16], F16,
                            kind="ExternalInput").ap()
    bcol_d = nc.dram_tensor("bcol", [128, OB], F32,
                            kind="ExternalInput").ap()
    m_d = nc.dram_tensor("m_out", [T, O_SHARD, B], F16,
                         kind="ExternalOutput").ap()
    s_d = nc.dram_tensor("s_out", [T, O_SHARD, B], F8,
                         kind="ExternalOutput").ap()

    with TileContext(nc) as tc:
        with tc.tile_pool(name="wpool", bufs=1) as wpool, \
             tc.tile_pool(name="xhpool", bufs=2) as xhpool, \
             tc.tile_pool(name="cpool", bufs=1) as cpool, \
             tc.tile_pool(name="mpool", bufs=4) as mpool, \
             tc.tile_pool(name="opool", bufs=4) as opool, \
             tc.tile_pool(name="spool", bufs=4) as spool, \
             tc.tile_pool(name="upool", bufs=3) as upool, \
             tc.tile_pool(name="psum", bufs=PSB, space="PSUM") as psum_pool:

            whs = wpool.tile([128, KT * O_SHARD], F16, name="whs")
            bcol_t = cpool.tile([128, OB], F32)
            d_t = [cpool.tile([128, B], F32, name=f"d{ob}") for ob in range(OB)]
            warm_t = cpool.tile([128, 2 * B], F16, name="warm")
            dump_t = cpool.tile([128, 16], F32, name="dump")

            xsz = NF16 // XCH
            wsz = KT * O_SHARD // WCH

            # pass-0 load, interleaved in PE consumption order (k-outer
            # pass 0 walks kt 0..31 across o-blocks, so pair (wh16, xh16)
            # chunks by kt range).
            xh0 = xhpool.tile([128, NF16], F16, tag="xh")
            nc.sync.dma_start(out=bcol_t, in_=bcol_d)
            for c in range(max(XCH, WCH)):
                if c < WCH:
                    wsl = slice(c * wsz, (c + 1) * wsz)
                    nc.sync.dma_start(out=whs[:, wsl], in_=wh16_d[:, wsl])
                if c < XCH:
                    xsl = slice(c * xsz, (c + 1) * xsz)
                    nc.sync.dma_start(out=xh0[:, xsl], in_=xh16_d[0][:, xsl])

            wh_k = whs.rearrange("p (kt o) -> p kt o", kt=KT)

            def warmup():
                if WARM_MM <= 0:
                    return
                nc.gpsimd.memset(warm_t, 0.0)
                wp = psum_pool.tile([128, 2 * B], F32, tag="hi3",
                                    name="warmpsum")
                for i in range(WARM_MM):
                    nc.tensor.matmul(wp, warm_t[:, :128], warm_t,
                                     start=(i == 0), stop=(i == WARM_MM - 1))
                nc.vector.tensor_copy(out=dump_t, in_=wp[:, :16])

            def body(first=None, warm=False):
                for ob in range(OB):
                    nc.vector.memset(d_t[ob], 0.0)
                    nc.scalar.add(d_t[ob], d_t[ob], bcol_t[:, ob:ob + 1])
                if warm:
                    warmup()
                for p in range(NPASS):
                    if p == 0 and first is not None:
                        xh = first
                    else:
                        xh = xhpool.tile([128, NF16], F16, tag="xh")
                        for c in range(XCH):
                            xsl = slice(c * xsz, (c + 1) * xsz)
                            nc.sync.dma_start(out=xh[:, xsl],
                                              in_=xh16_d[p][:, xsl])

                    xh_kv = xh.rearrange("p (kt n) -> p kt n", kt=KT)

                    hi_t = [psum_pool.tile([128, 2 * B], F32, tag=f"hi{ob}",
                                           name=f"hi{p}_{ob}")
                            for ob in range(OB)]

                    def mm_hi(ob, kt):
                        osl = slice(ob * 128, (ob + 1) * 128)
                        nc.tensor.matmul(hi_t[ob], wh_k[:, kt, osl],
                                         xh_kv[:, kt, :],
                                         start=(kt == 0), stop=(kt == KT - 1))

                    def drain(ob):
                        osl = slice(ob * 128, (ob + 1) * 128)
                        for ti in range(2):
                            t = 2 * p + ti
                            bsl = slice(ti * B, (ti + 1) * B)
                            m_sb = mpool.tile([128, B], F32, tag="m")
                            nc.vector.tensor_add(out=m_sb,
                                                 in0=hi_t[ob][:, bsl],
                                                 in1=d_t[ob])
                            s_sb = spool.tile([128, B], F8, tag="s")
                            m16_sb = opool.tile([128, B], F16, tag="m16")
                            if OFFLOAD:
                                nc.gpsimd.tensor_scalar(
                                    out=s_sb, in0=m_sb, scalar1=M_TH,
                                    scalar2=None, op0=mybir.AluOpType.is_ge)
                                nc.scalar.copy(m16_sb, m_sb)
                            else:
                                nc.vector.tensor_scalar(
                                    out=s_sb, in0=m_sb, scalar1=M_TH,
                                    scalar2=None, op0=mybir.AluOpType.is_ge)
                                nc.vector.tensor_copy(out=m16_sb, in_=m_sb)
                            nc.sync.dma_start(out=m_d[t, osl, :], in_=m16_sb)
                            nc.sync.dma_start(out=s_d[t, osl, :], in_=s_sb)
                            if t < T - 1:  # d is dead after the last step
                                u_sb = upool.tile([128, B], F32, tag="u")
                                nc.vector.tensor_scalar(
                                    out=u_sb, in0=m_sb, scalar1=M_TH,
                                    scalar2=ALPHA,
                                    op0=mybir.AluOpType.is_lt,
                                    op1=mybir.AluOpType.mult)
                                nc.vector.tensor_mul(out=u_sb, in0=m_sb,
                                                     in1=u_sb)
                                nc.scalar.add(d_t[ob], u_sb,
                                              bcol_t[:, ob:ob + 1])

                    if p == 0:
                        # k-outer while the cold DMA streams in, then
                        # ob-sequential so the chains finish staggered
                        # and drains overlap the tail of the pass.
                        KSPLIT = 24
                        for kt in range(KSPLIT):
                            for ob in range(OB):
                                mm_hi(ob, kt)
                        for ob in range(OB):
                            for kt in range(KSPLIT, KT):
                                mm_hi(ob, kt)
                            drain(ob)
                    else:
                        for ob in range(OB):
                            for kt in range(KT):
                                mm_hi(ob, kt)
                            drain(ob)

            if reps == 1:
                body(xh0, warm=True)
            elif os.environ.get("BMU_UNROLL") == "1":
                body(xh0, warm=True)
                for _ in range(reps - 1):
                    body()
            else:
                body(xh0, warm=True)
                with tc.For_i(0, reps - 1, 1):
                    body()

    nc.compile()
    return nc


def _get_nc():
    if "nc" not in _cache:
        _cache["nc"] = _build_kernel()
    return _cache["nc"]


def _prepare_in_maps(x: np.ndarray, W: np.ndarray, b: np.ndarray):
    xT = np.ascontiguousarray(x.transpose(0, 2, 1))  # [T, D_in, B]

    def ptile16(a):  # [T, D, B] -> [NPASS, 128, KT*2*B], [kt][ti][b]
        return np.ascontiguousarray(
            a.reshape(NPASS, 2, KT, 128, B).transpose(0, 3, 2, 1, 4)
            .reshape(NPASS, 128, KT * 2 * B))

    def wtile16(a):  # [D, O] -> [128, KT*O]
        o = a.shape[1]
        return np.ascontiguousarray(
            a.reshape(KT, 128, o).transpose(1, 0, 2).reshape(128, KT * o))

    xh16_t = ptile16(xT.astype(NP_F16))

    in_maps = []
    for c in range(N_CORES):
        sl = slice(c * O_SHARD, (c + 1) * O_SHARD)
        Wt = np.ascontiguousarray(W[sl, :].T)  # [D, O]
        bcol = np.ascontiguousarray(
            b[sl].astype(np.float32).reshape(OB, 128).T)  # [128, OB]
        in_maps.append({
            "wh16": wtile16(Wt.astype(NP_F16)),
            "xh16": xh16_t,
            "bcol": bcol,
        })
    return in_maps


def kernel(x: np.ndarray, W: np.ndarray, b: np.ndarray):
    x = np.asarray(x, dtype=np.float32)
    W = np.asarray(W, dtype=np.float32)
    b = np.asarray(b, dtype=np.float32)
    nc = _get_nc()
    in_maps = _prepare_in_maps(x, W, b)
    res = None
    for attempt in range(3):
        try:
            res = run_bass_kernel_spmd(nc, in_maps,
                                       core_ids=list(range(N_CORES)))
            break
        except Exception:
            # transient device errors (NRT INTERNAL/UNRECOVERABLE) clear
            # on retry; re-raise only if persistent
            if attempt == 2:
                raise
    m = np.empty((T, B, D), dtype=np.float32)
    s = np.empty((T, B, D), dtype=np.float32)
    for c in range(N_CORES):
        sl = slice(c * O_SHARD, (c + 1) * O_SHARD)
        m[:, :, sl] = res.results[c]["m_out"].astype(np.float32) \
            .transpose(0, 2, 1)
        s[:, :, sl] = res.results[c]["s_out"].astype(np.float32) \
            .transpose(0, 2, 1)
    return (m, s)


# revision 16
# speedup vs baseline: 1.6976x; 1.0144x over previous
"""Trainium2 Bass kernel for nn_BoundMemUpdate (spiking membrane update).

Computes, for x:[T,B,D], W:[D,D], b:[D]:
    mm[t] = x[t] @ W.T + b
    m[t] = mm[t] + m[t-1] * (1 - s[t-1]) * 0.5
    s[t] = (m[t] >= 1.0)
Returns (m, s), each [T, B, D] float32.

Sharding: output-dim (D_out) sharded 8 ways across cores (512 each);
x replicated, W/b sharded by rows. The recurrence is per-neuron
elementwise, so no cross-core communication is needed.

Matmul: single-term fp16. x and W are rounded to fp16; the PE forms
exact fp16 products with fp32 accumulation, so the only error is the
input rounding (~2^-11 relative per operand). On the fixed seed-0
problem instance this measures 6.5e-3 concatenated relative error
(506 spike flips of 8.4M) against the fp32 reference -- 3x inside
the 2e-2 gate. W is the PE-stationary operand and x the moving one,
giving output tiles [o_part, (t,b)] and 512 matmuls of 518 cycles
per core (~111 us of PE streaming at 2.4 GHz).

Schedule: 4 passes over t-pairs, 4 o-block PSUM chains per pass with
K=4096 contraction, double-generation PSUM banks (8 total) so pass
boundaries never wait on drains. The vector engine fuses the drain
with the temporal recurrence; the bias enters through the scalar
engine's per-partition bias port (d-state starts at b and is
re-biased every step). x is stored pass-interleaved in DRAM
([pass][128][kt][t][b]) so every DMA is contiguous on both sides;
pass 0 streams k-outer so the PE consumes (W, x) chunks as they
land, and a short warmup matmul chain ramps the PE clock during the
cold 8 MB load. Outputs go out as m:fp16 / s:fp8 in [t, o, b]
layout; the host widens and transposes during the final gather.
"""
import os
import numpy as np

import concourse.bass as bass
import concourse.mybir as mybir
from concourse import bacc
from concourse.tile import TileContext
from concourse.bass_utils import run_bass_kernel_spmd

T, B, D = 8, 256, 4096
N_CORES = 8
O_SHARD = D // N_CORES   # 512
KT = D // 128            # 32 fp16 k-tiles
OB = O_SHARD // 128      # 4 output blocks
NPASS = T // 2           # 4 t-pair passes
ALPHA = 0.5
M_TH = 1.0
WARM_MM = int(os.environ.get("BMU_WARM", "32"))
PSB = int(os.environ.get("BMU_PSB", "1"))       # psum pool bufs
XCH = int(os.environ.get("BMU_XCH", "8"))       # x DMA chunks per pass
WCH = int(os.environ.get("BMU_WCH", "8"))       # W DMA chunks
OFFLOAD = os.environ.get("BMU_OFFLOAD", "0") == "1"  # s8->gpsimd, m16->scalar

F16 = mybir.dt.float16
F8 = mybir.dt.float8e4
F32 = mybir.dt.float32
NP_F16 = np.float16

_cache = {}


def _build_kernel(reps: int = 1):
    nc = bacc.Bacc("TRN2", target_bir_lowering=False, debug=False,
                   num_devices=N_CORES)

    NF16 = KT * 2 * B        # per-pass x free size (16384)

    wh16_d = nc.dram_tensor("wh16", [128, KT * O_SHARD], F16,
                            kind="ExternalInput").ap()
    xh16_d = nc.dram_tensor("xh16", [NPASS, 128, NF16], F16,
                            kind="ExternalInput").ap()
    bcol_d = nc.dram_tensor("bcol", [128, OB], F32,
                            kind="ExternalInput").ap()
    m_d = nc.dram_tensor("m_out", [T, O_SHARD, B], F16,
                         kind="ExternalOutput").ap()
    s_d = nc.dram_tensor("s_out", [T, O_SHARD, B], F8,
                         kind="ExternalOutput").ap()

    with TileContext(nc) as tc:
        with tc.tile_pool(name="wpool", bufs=1) as wpool, \
             tc.tile_pool(name="xhpool", bufs=2) as xhpool, \
             tc.tile_pool(name="cpool", bufs=1) as cpool, \
             tc.tile_pool(name="mpool", bufs=4) as mpool, \
             tc.tile_pool(name="opool", bufs=4) as opool, \
             tc.tile_pool(name="spool", bufs=4) as spool, \
             tc.tile_pool(name="upool", bufs=3) as upool, \
             tc.tile_pool(name="psum", bufs=PSB, space="PSUM") as psum_pool:

            whs = wpool.tile([128, KT * O_SHARD], F16, name="whs")
            bcol_t = cpool.tile([128, OB], F32)
            d_t = [cpool.tile([128, B], F32, name=f"d{ob}") for ob in range(OB)]
            warm_t = cpool.tile([128, 2 * B], F16, name="warm")
            dump_t = cpool.tile([128, 16], F32, name="dump")

            xsz = NF16 // XCH
            wsz = KT * O_SHARD // WCH

            # pass-0 load, interleaved in PE consumption order (k-outer
            # pass 0 walks kt 0..31 across o-blocks, so pair (wh16, xh16)
            # chunks by kt range).
            xh0 = xhpool.tile([128, NF16], F16, tag="xh")
            nc.scalar.dma_start(out=bcol_t, in_=bcol_d)
            # W rides the Activation-engine DGE queue, x the SP queue --
            # two hardware rings drain the cold start in parallel
            for c in range(max(XCH, WCH)):
                if c < WCH:
                    wsl = slice(c * wsz, (c + 1) * wsz)
                    nc.scalar.dma_start(out=whs[:, wsl], in_=wh16_d[:, wsl])
                if c < XCH:
                    xsl = slice(c * xsz, (c + 1) * xsz)
                    nc.sync.dma_start(out=xh0[:, xsl], in_=xh16_d[0][:, xsl])

            wh_k = whs.rearrange("p (kt o) -> p kt o", kt=KT)

            def warmup():
                if WARM_MM <= 0:
                    return
                nc.gpsimd.memset(warm_t, 0.0)
                wp = psum_pool.tile([128, 2 * B], F32, tag="hi3",
                                    name="warmpsum")
                for i in range(WARM_MM):
                    nc.tensor.matmul(wp, warm_t[:, :128], warm_t,
                                     start=(i == 0), stop=(i == WARM_MM - 1))
                nc.vector.tensor_copy(out=dump_t, in_=wp[:, :16])

            def body(first=None, warm=False):
                for ob in range(OB):
                    nc.vector.memset(d_t[ob], 0.0)
                    nc.scalar.add(d_t[ob], d_t[ob], bcol_t[:, ob:ob + 1])
                if warm:
                    warmup()
                for p in range(NPASS):
                    if p == 0 and first is not None:
                        xh = first
                    else:
                        xh = xhpool.tile([128, NF16], F16, tag="xh")
                        for c in range(XCH):
                            xsl = slice(c * xsz, (c + 1) * xsz)
                            eng = nc.sync if c % 2 == 0 else nc.scalar
                            eng.dma_start(out=xh[:, xsl],
                                          in_=xh16_d[p][:, xsl])

                    xh_kv = xh.rearrange("p (kt n) -> p kt n", kt=KT)

                    hi_t = [psum_pool.tile([128, 2 * B], F32, tag=f"hi{ob}",
                                           name=f"hi{p}_{ob}")
                            for ob in range(OB)]

                    def mm_hi(ob, kt):
                        osl = slice(ob * 128, (ob + 1) * 128)
                        nc.tensor.matmul(hi_t[ob], wh_k[:, kt, osl],
                                         xh_kv[:, kt, :],
                                         start=(kt == 0), stop=(kt == KT - 1))

                    def drain(ob):
                        osl = slice(ob * 128, (ob + 1) * 128)
                        for ti in range(2):
                            t = 2 * p + ti
                            bsl = slice(ti * B, (ti + 1) * B)
                            m_sb = mpool.tile([128, B], F32, tag="m")
                            nc.vector.tensor_add(out=m_sb,
                                                 in0=hi_t[ob][:, bsl],
                                                 in1=d_t[ob])
                            s_sb = spool.tile([128, B], F8, tag="s")
                            m16_sb = opool.tile([128, B], F16, tag="m16")
                            if OFFLOAD:
                                nc.gpsimd.tensor_scalar(
                                    out=s_sb, in0=m_sb, scalar1=M_TH,
                                    scalar2=None, op0=mybir.AluOpType.is_ge)
                                nc.scalar.copy(m16_sb, m_sb)
                            else:
                                nc.vector.tensor_scalar(
                                    out=s_sb, in0=m_sb, scalar1=M_TH,
                                    scalar2=None, op0=mybir.AluOpType.is_ge)
                                nc.vector.tensor_copy(out=m16_sb, in_=m_sb)
                            nc.sync.dma_start(out=m_d[t, osl, :], in_=m16_sb)
                            nc.sync.dma_start(out=s_d[t, osl, :], in_=s_sb)
                            if t < T - 1:  # d is dead after the last step
                                u_sb = upool.tile([128, B], F32, tag="u")
                                nc.vector.tensor_scalar(
                                    out=u_sb, in0=m_sb, scalar1=M_TH,
                                    scalar2=ALPHA,
                                    op0=mybir.AluOpType.is_lt,
                                    op1=mybir.AluOpType.mult)
                                nc.vector.tensor_mul(out=u_sb, in0=m_sb,
                                                     in1=u_sb)
                                nc.scalar.add(d_t[ob], u_sb,
                                              bcol_t[:, ob:ob + 1])

                    if p == 0:
                        # k-outer while the cold DMA streams in, then
                        # ob-sequential so the chains finish staggered
                        # and drains overlap the tail of the pass.
                        KSPLIT = 24
                        for kt in range(KSPLIT):
                            for ob in range(OB):
                                mm_hi(ob, kt)
                        for ob in range(OB):
                            for kt in range(KSPLIT, KT):
                                mm_hi(ob, kt)
                            drain(ob)
                    else:
                        for ob in range(OB):
                            for kt in range(KT):
                                mm_hi(ob, kt)
                            drain(ob)

            if reps == 1:
                body(xh0, warm=True)
            elif os.environ.get("BMU_UNROLL") == "1":
                body(xh0, warm=True)
                for _ in range(reps - 1):
                    body()
            else:
                body(xh0, warm=True)
                with tc.For_i(0, reps - 1, 1):
                    body()

    nc.compile()
    return nc


def _get_nc():
    if "nc" not in _cache:
        _cache["nc"] = _build_kernel()
    return _cache["nc"]


def _prepare_in_maps(x: np.ndarray, W: np.ndarray, b: np.ndarray):
    xT = np.ascontiguousarray(x.transpose(0, 2, 1))  # [T, D_in, B]

    def ptile16(a):  # [T, D, B] -> [NPASS, 128, KT*2*B], [kt][ti][b]
        return np.ascontiguousarray(
            a.reshape(NPASS, 2, KT, 128, B).transpose(0, 3, 2, 1, 4)
            .reshape(NPASS, 128, KT * 2 * B))

    def wtile16(a):  # [D, O] -> [128, KT*O]
        o = a.shape[1]
        return np.ascontiguousarray(
            a.reshape(KT, 128, o).transpose(1, 0, 2).reshape(128, KT * o))

    xh16_t = ptile16(xT.astype(NP_F16))

    in_maps = []
    for c in range(N_CORES):
        sl = slice(c * O_SHARD, (c + 1) * O_SHARD)
        Wt = np.ascontiguousarray(W[sl, :].T)  # [D, O]
        bcol = np.ascontiguousarray(
            b[sl].astype(np.float32).reshape(OB, 128).T)  # [128, OB]
        in_maps.append({
            "wh16": wtile16(Wt.astype(NP_F16)),
            "xh16": xh16_t,
            "bcol": bcol,
        })
    return in_maps


def kernel(x: np.ndarray, W: np.ndarray, b: np.ndarray):
    x = np.asarray(x, dtype=np.float32)
    W = np.asarray(W, dtype=np.float32)
    b = np.asarray(b, dtype=np.float32)
    nc = _get_nc()
    in_maps = _prepare_in_maps(x, W, b)
    res = None
    for attempt in range(3):
        try:
            res = run_bass_kernel_spmd(nc, in_maps,
                                       core_ids=list(range(N_CORES)))
            break
        except Exception:
            # transient device errors (NRT INTERNAL/UNRECOVERABLE) clear
            # on retry; re-raise only if persistent
            if attempt == 2:
                raise
    m = np.empty((T, B, D), dtype=np.float32)
    s = np.empty((T, B, D), dtype=np.float32)
    for c in range(N_CORES):
        sl = slice(c * O_SHARD, (c + 1) * O_SHARD)
        m[:, :, sl] = res.results[c]["m_out"].astype(np.float32) \
            .transpose(0, 2, 1)
        s[:, :, sl] = res.results[c]["s_out"].astype(np.float32) \
            .transpose(0, 2, 1)
    return (m, s)
